# revision 10
# baseline (speedup 1.0000x reference)
"""SSD-multibox-style loss (classification CE + hard-negative mining + smooth-L1)
on 8 Trainium2 NeuronCores.

Strategy (data-parallel over batch):
  - B=16 rows are sharded 2 rows/core across 8 cores. Each core streams its
    y_pre / y_batch rows from HBM once (memory-bound target) and computes, per
    row, a small set of sufficient statistics:
        num_pos            = #positive anchors
        sum_plz            = sum over anchors of pos * logZ
        sum_dot            = sum over anchors/classes of c_pre * c_hat
                             (== sum over positives of c_pre[label])
        sq                 = sum over positives of |b_pre-b_hat|^2
        r2a+r2b            = sum over positives of relu(+-d - 1)^2
        t_star, S_rel      = hard-negative-mining threshold + partial sum
  - Hard-negative mining: selecting the num_neg negatives with lowest
    background confidence == selecting the num_neg LARGEST ce0 = logZ - x0.
    The kernel finds t ~= the k-th largest value by bisecting on exact counts
    (11 iterations), then uses the dual form
        sum_topk(ce0) == sum(relu(ce0 - t)) + k*t          (exact if count==k,
    error bounded by |count-k| * bracket_width ~ 1e-5 otherwise).
  - The host combines the 16 rows' statistics into the 3 scalar losses
    (the "all-reduce" of the sharding hint, done on 16 tiny values).

Numerics: logZ is computed without max-subtraction (inputs are ~N(0,1), so
exp() cannot overflow in fp32); smooth-L1 uses
    huber(d) = 0.5*d^2 - 0.5*(relu(d-1)^2 + relu(-d-1)^2).
"""

import sys

for _p in ("/opt/trn_rl_repo", "/root/.axon_site/_ro/trn_rl_repo"):
    if _p not in sys.path:
        sys.path.insert(0, _p)

from contextlib import ExitStack

import numpy as np

NUM_CLASSES = 6
BETA = 0.5
B, A, CH = 16, 131072, 10
N_CORES = 8
ROWS = B // N_CORES          # rows per core
P = 128                      # partitions
NCHUNK = 4                   # chunks per row
NITER = 11                   # bisection iterations
W0 = 8.0                     # initial bracket width for t in (0, 8)
NOUT = 8                     # per-row output stats

_QN = ["np", "plz", "dot", "sq", "r2a", "r2b"]  # accumulated quantities


def build_program(A=A, rows=ROWS, nchunk=NCHUNK, niter=NITER, w0=W0):
    """Build (and compile) the per-core Bass program. Returns nc."""
    import concourse.bacc as bacc
    import concourse.tile as tile
    from concourse import bass_isa, mybir

    f32 = mybir.dt.float32
    AL = mybir.AluOpType
    AF = mybir.ActivationFunctionType
    AX = mybir.AxisListType
    C = NUM_CLASSES

    NPP = A // P             # anchors per partition per row
    CW = NPP // nchunk       # anchors per partition per chunk

    nc = bacc.Bacc("TRN2", target_bir_lowering=False, debug=False)
    ypre = nc.dram_tensor("y_pre", [rows, A, CH], f32, kind="ExternalInput").ap()
    ybat = nc.dram_tensor("y_batch", [rows, A, CH], f32, kind="ExternalInput").ap()
    out = nc.dram_tensor("out", [rows, NOUT], f32, kind="ExternalOutput").ap()

    with tile.TileContext(nc) as tc, ExitStack() as ctx:
        inp = ctx.enter_context(tc.tile_pool(name="inp", bufs=3))
        exl = ctx.enter_context(tc.tile_pool(name="exl", bufs=2))
        scr = ctx.enter_context(tc.tile_pool(name="scr", bufs=3))
        boxp = ctx.enter_context(tc.tile_pool(name="boxp", bufs=2))
        dmy = ctx.enter_context(tc.tile_pool(name="dmy", bufs=5))
        per = ctx.enter_context(tc.tile_pool(name="per", bufs=1))
        bis = ctx.enter_context(tc.tile_pool(name="bis", bufs=3))

        negone = per.tile([P, 1], f32, name="negone", tag="negone")
        nc.vector.memset(negone[:], -1.0)
        z_t = [per.tile([P, NPP], f32, name=f"z{r}", tag=f"z{r}") for r in range(rows)]
        acc = {
            (r, q): per.tile([P, nchunk], f32, name=f"acc_{r}_{q}", tag=f"acc_{r}_{q}")
            for r in range(rows)
            for q in _QN
        }

        # ---------------- per-row bisection (emitted right after the row's
        # chunks so row 0's serial bisection overlaps row 1's streaming) ----
        RO = bass_isa.ReduceOp
        nlo_t = {}

        def emit_bisect(r):
            npc = bis.tile([P, 1], f32, name=f"npc{r}", tag=f"npc{r}")
            nc.vector.tensor_reduce(npc[:], acc[(r, "np")][:], AX.X, AL.add)
            npall = bis.tile([P, 1], f32, name=f"npall{r}", tag=f"npall{r}")
            nc.gpsimd.partition_all_reduce(npall[:], npc[:], P, RO.add)
            k3 = bis.tile([P, 1], f32, name=f"k3{r}", tag=f"k3{r}")
            nc.vector.tensor_scalar(k3[:], npall[:], 3.0, None, AL.mult)
            nna = bis.tile([P, 1], f32, name=f"nna{r}", tag=f"nna{r}")
            nc.vector.tensor_scalar(nna[:], npall[:], -1.0, float(A), AL.mult, AL.add)
            kk = bis.tile([P, 1], f32, name=f"kk{r}", tag=f"kk{r}")
            nc.vector.tensor_tensor(kk[:], k3[:], nna[:], AL.min)
            nlo = bis.tile([P, 1], f32, name=f"nlo{r}", tag=f"nlo{r}")
            nc.vector.memset(nlo[:], 0.0)
            for i in range(niter):
                w = w0 / (2.0 ** (i + 1))
                nt = bis.tile([P, 1], f32, name=f"nt{r}", tag=f"nt{r}")
                nc.vector.tensor_scalar(nt[:], nlo[:], -w, None, AL.add)
                cp = bis.tile([P, 1], f32, name=f"cp{r}", tag=f"cp{r}")
                cd = dmy.tile([P, NPP], f32, name=f"cd{r}", tag="cdj")
                nc.vector.tensor_scalar(
                    cd[:], z_t[r][:], nt[:], None, AL.is_lt, AL.add, accum_out=cp[:]
                )
                call = bis.tile([P, 1], f32, name=f"call{r}", tag=f"call{r}")
                nc.gpsimd.partition_all_reduce(call[:], cp[:], P, RO.add)
                g = bis.tile([P, 1], f32, name=f"g{r}", tag=f"g{r}")
                nc.vector.tensor_tensor(g[:], call[:], kk[:], AL.is_ge)
                nlo2 = bis.tile([P, 1], f32, name=f"nlo{r}", tag=f"nlo{r}")
                nc.vector.scalar_tensor_tensor(
                    nlo2[:], g[:], -w, nlo[:], AL.mult, AL.add
                )
                nlo = nlo2
            nlo_t[r] = nlo

        # ---------------- streaming phase ----------------
        for r in range(rows):
            yp3 = ypre[r].rearrange("(p n) c -> p n c", p=P)   # [P, NPP, CH]
            yb3 = ybat[r].rearrange("(p n) c -> p n c", p=P)
            for j in range(nchunk):
                Tt = inp.tile([P, CW * CH], f32, name="T", tag="T")
                nc.sync.dma_start(
                    out=Tt[:].rearrange("p (n c) -> p n c", c=CH),
                    in_=yp3[:, j * CW:(j + 1) * CW, :],
                )
                Ut = inp.tile([P, CW * CH], f32, name="U", tag="U")
                nc.sync.dma_start(
                    out=Ut[:].rearrange("p (n c) -> p n c", c=CH),
                    in_=yb3[:, j * CW:(j + 1) * CW, :],
                )
                T3 = Tt[:].rearrange("p (n c) -> p n c", c=CH)
                U3 = Ut[:].rearrange("p (n c) -> p n c", c=CH)

                # E = sum_c exp(x_c); logZ = ln(E)
                ex = exl.tile([P, CW * C], f32, name="ex", tag="ex")
                ex3 = ex[:].rearrange("p (n c) -> p n c", c=C)
                nc.scalar.activation(ex3, T3[:, :, 0:C], AF.Exp)
                E = scr.tile([P, CW], f32, name="E", tag="E")
                nc.vector.tensor_reduce(E[:], ex3, AX.X, AL.add)
                lz = scr.tile([P, CW], f32, name="lz", tag="lz")
                nc.scalar.activation(lz[:], E[:], AF.Ln)

                # pos = sum_c c_hat (exactly 1.0 / 0.0)
                pos = scr.tile([P, CW], f32, name="pos", tag="pos")
                nc.vector.tensor_reduce(pos[:], U3[:, :, 0:C], AX.X, AL.add)
                # num_pos partial
                nc.vector.tensor_reduce(
                    acc[(r, "np")][:, j:j + 1], pos[:], AX.X, AL.add
                )
                # sum_dot partial (GPSIMD): sum(c_pre * c_hat)
                nc.vector.scalar_tensor_tensor(
                    ex3, T3[:, :, 0:C], 1.0, U3[:, :, 0:C],
                    AL.mult, AL.mult,
                    accum_out=acc[(r, "dot")][:, j:j + 1],
                )
                # sum_plz partial: sum(pos * logZ)
                pd = dmy.tile([P, CW], f32, name="pd", tag="junk")
                nc.vector.scalar_tensor_tensor(
                    pd[:], pos[:], 1.0, lz[:], AL.mult, AL.mult,
                    accum_out=acc[(r, "plz")][:, j:j + 1],
                )

                # z = (pos - 1) * ce0  (0 for positives, -ce0 for negatives)
                ce0 = scr.tile([P, CW], f32, name="ce0", tag="ce0")
                t0 = T3[:, :, 0:1].rearrange("p n c -> p (n c)")
                nc.vector.tensor_tensor(ce0[:], lz[:], t0, AL.subtract)
                nc.vector.scalar_tensor_tensor(
                    z_t[r][:, j * CW:(j + 1) * CW], pos[:], 1.0, ce0[:],
                    AL.subtract, AL.mult,
                )

                # box terms
                d = boxp.tile([P, CW * 4], f32, name="d", tag="d")
                d3 = d[:].rearrange("p (n o) -> p n o", o=4)
                nc.gpsimd.tensor_tensor(d3, T3[:, :, C:CH], U3[:, :, C:CH], AL.subtract)
                posb = pos[:].rearrange("p (n o) -> p n o", o=1).broadcast_to([P, CW, 4])
                dm = boxp.tile([P, CW * 4], f32, name="dm", tag="dm")
                dm3 = dm[:].rearrange("p (n o) -> p n o", o=4)
                nc.gpsimd.tensor_tensor(dm3, d3, posb, AL.mult)
                sqo = dmy.tile([P, CW * 4], f32, name="sqo", tag="junk")
                nc.scalar.activation(
                    sqo[:], dm[:], AF.Square, accum_out=acc[(r, "sq")][:, j:j + 1]
                )
                ra = boxp.tile([P, CW * 4], f32, name="ra", tag="ra")
                nc.scalar.activation(ra[:], dm[:], AF.Relu, bias=negone[:], scale=1.0)
                rb = boxp.tile([P, CW * 4], f32, name="rb", tag="rb")
                nc.scalar.activation(rb[:], dm[:], AF.Relu, bias=negone[:], scale=-1.0)
                sqa = dmy.tile([P, CW * 4], f32, name="sqa", tag="junk")
                nc.scalar.activation(
                    sqa[:], ra[:], AF.Square, accum_out=acc[(r, "r2a")][:, j:j + 1]
                )
                sqb = dmy.tile([P, CW * 4], f32, name="sqb", tag="junk")
                nc.scalar.activation(
                    sqb[:], rb[:], AF.Square, accum_out=acc[(r, "r2b")][:, j:j + 1]
                )
            emit_bisect(r)

        wf = w0 / (2.0 ** niter)
        for r in range(rows):
            nts = bis.tile([P, 1], f32, name=f"nts{r}", tag=f"nts{r}")
            nc.vector.tensor_scalar(nts[:], nlo_t[r][:], -wf * 0.5, None, AL.add)
            sro = dmy.tile([P, NPP], f32, name=f"sro{r}", tag="cdj")
            sp = bis.tile([P, 1], f32, name=f"sp{r}", tag=f"sp{r}")
            nc.scalar.activation(
                sro[:], z_t[r][:], AF.Relu, bias=nts[:], scale=-1.0, accum_out=sp[:]
            )
            RO = bass_isa.ReduceOp
            sall = bis.tile([P, 1], f32, name=f"sall{r}", tag=f"sall{r}")
            nc.gpsimd.partition_all_reduce(sall[:], sp[:], P, RO.add)
            tst = bis.tile([P, 1], f32, name=f"tst{r}", tag=f"tst{r}")
            nc.vector.tensor_scalar(tst[:], nts[:], -1.0, None, AL.mult)

            stage = per.tile([P, NOUT], f32, name=f"stage{r}", tag=f"stage{r}")
            for qi, q in enumerate(_QN):
                nc.vector.tensor_reduce(stage[:, qi:qi + 1], acc[(r, q)][:], AX.X, AL.add)
            stg = per.tile([P, NOUT], f32, name=f"stg{r}", tag=f"stg{r}")
            nc.gpsimd.partition_all_reduce(stg[:, 0:6], stage[:, 0:6], P, RO.add)
            nc.vector.tensor_copy(stg[:, 6:7], tst[:])
            nc.vector.tensor_copy(stg[:, 7:8], sall[:])
            nc.sync.dma_start(out=out[r:r + 1, :], in_=stg[0:1, :])

    nc.compile()
    return nc


_PROGRAM = None


def _get_program():
    global _PROGRAM
    if _PROGRAM is None:
        _PROGRAM = build_program()
    return _PROGRAM


def assemble(stats):
    """Combine per-row stats [B, NOUT] -> (total, L_class, L_box) in float32."""
    f32 = np.float32
    s = np.asarray(stats, dtype=np.float32)
    num_pos = s[:, 0]
    pos_loss = s[:, 1] - s[:, 2]
    sq, r2a, r2b = s[:, 3], s[:, 4], s[:, 5]
    t_star, s_rel = s[:, 6], s[:, 7]

    num_neg_avail = f32(A) - num_pos
    num_neg = np.minimum(3 * num_pos, num_neg_avail)
    neg_loss = s_rel + num_neg * t_star
    has_pos = num_pos > 0
    has_neg = num_neg_avail > 0
    denom = np.where(has_neg, num_neg + num_pos, num_pos).astype(np.float32)
    per_sample = np.where(has_neg, pos_loss + neg_loss, pos_loss) / np.maximum(denom, 1.0)
    per_sample = np.where(has_pos, per_sample, 0.0).astype(np.float32)
    n_valid = int(np.sum(has_pos))
    L_class = f32(np.sum(per_sample) / max(n_valid, 1)) if n_valid > 0 else f32(0.0)

    L_box_sum = f32(0.5) * (np.sum(sq) - (np.sum(r2a) + np.sum(r2b)))
    total_pos = np.sum(num_pos)
    L_box = f32(L_box_sum / (total_pos + f32(1e-6))) if total_pos > 0 else f32(0.0)
    total = f32(L_class + f32(BETA) * L_box)
    return total, f32(L_class), L_box


def kernel(y_pre: np.ndarray, y_batch: np.ndarray):
    from concourse.bass_utils import run_bass_kernel_spmd

    y_pre = np.ascontiguousarray(np.asarray(y_pre, dtype=np.float32))
    y_batch = np.ascontiguousarray(np.asarray(y_batch, dtype=np.float32))
    assert y_pre.shape == (B, A, CH) and y_batch.shape == (B, A, CH)

    nc = _get_program()
    in_maps = [
        {
            "y_pre": y_pre[c * ROWS:(c + 1) * ROWS],
            "y_batch": y_batch[c * ROWS:(c + 1) * ROWS],
        }
        for c in range(N_CORES)
    ]
    res = run_bass_kernel_spmd(nc, in_maps, list(range(N_CORES)))
    stats = np.concatenate([res.results[c]["out"] for c in range(N_CORES)], axis=0)
    total, l_class, l_box = assemble(stats)
    return (np.float32(total), np.float32(l_class), np.float32(l_box))
